# revision 13
# baseline (speedup 1.0000x reference)
"""Trainium2 Bass kernel for dense_cnn problem.

Math (per batch element n, C=128 channels, H=W=56, G=8):
  t1 = conv_h(x, w1)          5-tap conv over H with full channel mixing
  t3 = dwconv_h(t1, w3)       3-tap depthwise conv over H
  t4[g] = sum_{c,k} x[c, h, w+2k-2] * w4[c,k,g]   (3 width taps, dil 2)
  out[c] = t3[c] * t4[c % 8]

Device strategy (data-parallel, 4 batch elems per core across 8 cores):
  - Fold t3 = w3 (*) w1 (*) x into ONE 7-tap H-conv with combined weights
    wc[f, ci, co] = sum_{d+e=f} w3[co,d] * w1[co,ci,e]  -> PE matmuls.
    The fold is only exact where the intermediate t1 index stays in
    [0,56); 4 tiny correction matmuls fix output rows 0 and 55.
  - Broadcast w4 across the 16 channel groups on the host:
    w4b[ci, k, co] = w4[ci, k, co % 8] -> t4 computed directly at 128
    output channels (3 PE matmuls), so the final combine is elementwise.
  - Matmuls in bf16 (fp32 matmul lowers to a LOW_HIGH pair at <half
    throughput); accumulation stays fp32 in PSUM.
  - x stays unpadded/contiguous in SBUF; boundary taps are emitted as
    clipped matmuls over row/col sub-ranges (the skipped regions keep
    the value accumulated by the always-full center tap).
  - Per 8-row chunk: accumulate psA (t3) and psB (t4); ScalarE copies
    psB->SBUF, VectorE multiplies into a per-n output buffer; one big
    DMA per n for input and output (avoids tiny-packet DMA).
"""

import sys

sys.path.insert(0, "/opt/trn_rl_repo")

import ml_dtypes
import numpy as np

import concourse.bacc as bacc
import concourse.bass as bass
import concourse.mybir as mybir
import concourse.tile as tile
from concourse import bass_utils

N, C, H, W, G = 32, 128, 56, 56, 8
NCORES = 8
NPC = N // NCORES  # batch elems per core
CH = 8             # H rows per chunk
NCHUNK = H // CH

F32 = mybir.dt.float32
BF16 = mybir.dt.bfloat16

TRACE = False
TRACE_DIR = None
LAST_EXEC_NS = None
LAST_RESULTS = None

_COMPILED = None


def _enable_trace_hook():
    """The agent image's ``antenv`` lacks ``axon_hooks``, so the boot-time
    NTFF hook registration silently degraded. Recreate the module and
    register the same ctypes-based hook; also skip the bucket upload."""
    import sys as _sys
    import types

    if "antenv.axon_hooks" not in _sys.modules:
        mod = types.ModuleType("antenv.axon_hooks")
        mod._hook = None

        def set_axon_ntff_profile_hook(h):
            mod._hook = h

        def get_axon_ntff_profile_hook():
            return mod._hook

        mod.set_axon_ntff_profile_hook = set_axon_ntff_profile_hook
        mod.get_axon_ntff_profile_hook = get_axon_ntff_profile_hook
        _sys.modules["antenv.axon_hooks"] = mod
        import antenv

        antenv.axon_hooks = mod

    from antenv.axon_hooks import get_axon_ntff_profile_hook as _get

    if _get() is None:
        from trn_agent_boot.trn_boot import _ntff_profile_via_ctypes

        hook = _ntff_profile_via_ctypes("/opt/axon/libaxon_pjrt.so")
        if hook is not None:
            _sys.modules["antenv.axon_hooks"].set_axon_ntff_profile_hook(hook)

    bass_utils.upload_artifacts = lambda tmpdir: f"local:{tmpdir}"


def _t3_matmuls(c, pa, xc, wc_t):
    """(lhsT, rhs, out) list accumulating the folded 7-tap conv for the
    8-row chunk c, with row clipping at the H borders plus the t1-clip
    correction taps. Output row o of the chunk reads x row 8c+o+f-3."""
    h0 = c * CH
    mms = []
    # f=3 covers the full chunk for every c -> emitted first (start=True)
    for f in (3, 0, 1, 2, 4, 5, 6):
        o_lo = max(0, 3 - f - h0)
        o_hi = min(CH, H + 3 - f - h0)
        if o_lo >= o_hi:
            continue
        r0 = h0 + o_lo + f - 3
        r1 = h0 + o_hi + f - 3
        mms.append((wc_t[:, f, :], xc[:, r0:r1, :], pa[:, o_lo:o_hi, :]))
    if c == 0:
        # fold wrongly includes t1[-1] at h=0: subtract w3[0]*w1[e]*x[e-3]
        for j in range(2):
            mms.append((wc_t[:, 7 + j, :], xc[:, j : j + 1, :], pa[:, 0:1, :]))
    if c == NCHUNK - 1:
        # fold wrongly includes t1[56] at h=55
        for j in range(2):
            mms.append(
                (wc_t[:, 9 + j, :], xc[:, 54 + j : 55 + j, :], pa[:, CH - 1 : CH, :])
            )
    return mms


def _t4_matmuls(c, pb, xc, w4_t):
    """t4 chunk: 3 width taps at offsets -2/0/+2, col-clipped at borders."""
    h0 = c * CH
    rows = xc[:, h0 : h0 + CH, :]
    return [
        (w4_t[:, 1, :], rows, pb[:]),                               # delta = 0
        (w4_t[:, 0, :], xc[:, h0 : h0 + CH, 0 : W - 2], pb[:, :, 2:W]),   # -2
        (w4_t[:, 2, :], xc[:, h0 : h0 + CH, 2:W], pb[:, :, 0 : W - 2]),   # +2
    ]


def _build():
    nc = bacc.Bacc(
        "TRN2",
        target_bir_lowering=False,
        debug=False,
        enable_asserts=False,
        num_devices=NCORES,
    )

    x_d = nc.dram_tensor("x_s", (NPC, C, H, W), BF16, kind="ExternalInput").ap()
    wc_d = nc.dram_tensor("wc", (C, 11, C), BF16, kind="ExternalInput").ap()
    w4_d = nc.dram_tensor("w4b", (C, 3, C), BF16, kind="ExternalInput").ap()
    out_d = nc.dram_tensor("out", (NPC, C, H, W), F32, kind="ExternalOutput").ap()

    with tile.TileContext(nc) as tc:
        with (
            tc.tile_pool(name="wpool", bufs=1) as wpool,
            tc.tile_pool(name="xpool", bufs=1) as xpool,
            tc.tile_pool(name="t4pool", bufs=3) as t4pool,
            tc.tile_pool(name="opool", bufs=3) as opool,
            tc.tile_pool(name="psA", bufs=2, space="PSUM") as papool,
            tc.tile_pool(name="psB", bufs=2, space="PSUM") as pbpool,
            tc.tile_pool(name="psD", bufs=1, space="PSUM") as pdpool,
        ):
            # Dummy matmuls on uninitialized SBUF while the first DMAs
            # stream in: PE_HAM ungates the 2.4 GHz clock only after
            # ~3.4us of sustained activity, so warm it up before the real
            # matmuls start. The garbage results land in a PSUM bank that
            # is never read.
            dmy = wpool.tile([C, 512], BF16)
            nc.gpsimd.memset(dmy[:], 0.0)
            dps = pdpool.tile([C, 512], F32)
            for _ in range(8):
                nc.tensor.matmul(
                    dps[:], lhsT=dmy[:, 0:C], rhs=dmy[:], start=True, stop=True
                )

            wc_t = wpool.tile([C, 11, C], BF16)
            nc.sync.dma_start(wc_t[:], wc_d[:])
            w4_t = wpool.tile([C, 3, C], BF16)

            xcs = []
            for n in range(NPC):
                xc = xpool.tile([C, H, W], BF16, name=f"xc{n}")
                xcs.append(xc)
            # first batch elem lands in two pieces so chunk-0 matmuls can
            # start before the whole tensor arrives; w4 is only needed a
            # few matmuls in, so it loads between the pieces
            nc.sync.dma_start(xcs[0][:, 0:28, :], x_d[0, :, 0:28, :])
            nc.sync.dma_start(w4_t[:], w4_d[:])
            nc.sync.dma_start(xcs[0][:, 28:H, :], x_d[0, :, 28:H, :])
            for n in range(1, NPC):
                nc.sync.dma_start(xcs[n][:], x_d[n])

            for n in range(NPC):
                xc = xcs[n]

                for c in range(NCHUNK):
                    h0 = c * CH
                    pa = papool.tile([C, CH, W], F32)
                    mms = _t3_matmuls(c, pa, xc, wc_t)
                    for i, (lhsT, rhs, outap) in enumerate(mms):
                        nc.tensor.matmul(
                            outap,
                            lhsT=lhsT,
                            rhs=rhs,
                            start=(i == 0),
                            stop=(i == len(mms) - 1),
                        )
                    pb = pbpool.tile([C, CH, W], F32)
                    mmsb = _t4_matmuls(c, pb, xc, w4_t)
                    for i, (lhsT, rhs, outap) in enumerate(mmsb):
                        nc.tensor.matmul(
                            outap,
                            lhsT=lhsT,
                            rhs=rhs,
                            start=(i == 0),
                            stop=(i == len(mmsb) - 1),
                        )
                    t4s = t4pool.tile([C, CH, W], F32)
                    nc.scalar.copy(t4s[:], pb[:])
                    ot = opool.tile([C, CH, W], F32)
                    nc.vector.tensor_mul(ot[:], pa[:], t4s[:])
                    nc.sync.dma_start(out_d[n, :, h0 : h0 + CH, :], ot[:])

    nc.compile()
    return nc


def _get_compiled():
    global _COMPILED
    if _COMPILED is None:
        _COMPILED = _build()
    return _COMPILED


def _prep_weights(w1, w3, w4):
    w1c = np.asarray(w1, dtype=np.float32)[:, :, :, 0]  # (co, ci, 5)
    w3c = np.asarray(w3, dtype=np.float32)[:, 0, :, 0]  # (co, 3)
    wc = np.zeros((C, 11, C), dtype=np.float32)         # (ci, tap, co)
    for d in range(3):
        for e in range(5):
            # wc[ci, d+e, co] += w1[co, ci, e] * w3[co, d]
            wc[:, d + e, :] += (w1c[:, :, e] * w3c[:, d][:, None]).T
    # border clip corrections (see _t3_matmuls): taps 7,8 fix h=0; 9,10 h=55
    for j, e in enumerate((3, 4)):
        wc[:, 7 + j, :] = -(w1c[:, :, e] * w3c[:, 0][:, None]).T
    for j, e in enumerate((0, 1)):
        wc[:, 9 + j, :] = -(w1c[:, :, e] * w3c[:, 2][:, None]).T
    w4c = np.asarray(w4, dtype=np.float32)[:, :, 0, :]  # (ci, k, g)
    w4b = np.ascontiguousarray(np.tile(w4c, (1, 1, C // G)))  # (ci, k, 128)
    bf = ml_dtypes.bfloat16
    return np.ascontiguousarray(wc).astype(bf), w4b.astype(bf)


def kernel(x, w1, w3, w4):
    global LAST_EXEC_NS, LAST_RESULTS
    nc = _get_compiled()
    xb = np.ascontiguousarray(np.asarray(x, dtype=np.float32)).astype(ml_dtypes.bfloat16)
    wc, w4b = _prep_weights(w1, w3, w4)

    in_maps = [
        {
            "x_s": np.ascontiguousarray(xb[i * NPC : (i + 1) * NPC]),
            "wc": wc,
            "w4b": w4b,
        }
        for i in range(NCORES)
    ]
    if TRACE:
        _enable_trace_hook()
    res = bass_utils.run_bass_kernel_spmd(
        nc,
        in_maps,
        core_ids=list(range(NCORES)),
        trace=TRACE,
        tmpdir=TRACE_DIR,
    )
    LAST_EXEC_NS = res.exec_time_ns
    LAST_RESULTS = res
    out = np.concatenate([res.results[i]["out"] for i in range(NCORES)], axis=0)
    return out
